# revision 7
# baseline (speedup 1.0000x reference)
"""ArcticMoE top-2 routing MoE — Trainium2 Bass kernel, expert-parallel over 8 cores.

Strategy:
  - Host computes the (tiny) router: logits = x @ gate_w.T, softmax, top-2,
    normalized combine weights, aux loss.  This is 0.03% of total FLOPs.
  - Expert parallelism: core e holds expert e's weights (pre-transposed/tiled
    to the exact SBUF layouts the matmuls need, cast to bf16 on host).
    Core e receives only the tokens routed to expert e (gathered, transposed),
    padded to a common capacity C so all 8 cores run one SPMD graph.
  - Device per core: hmidT = silu(W1T.T@XT) * (W3T.T@XT)  (tiles [f,c]),
    YT = W2T.T @ gT (tiles [h,c]) scaled by per-token combine weight.
  - Host scatter-adds each expert's [count, H] result into the [T, H] output.
"""

import os
import sys
import types

import numpy as np
import ml_dtypes

B, S, H, F, E, TOPK = 2, 2048, 2048, 4096, 8, 2
T = B * S
NH = H // 128   # 16 h-chunks
NF = F // 128   # 32 f-chunks
NHJ = H // 512  # 4 w2-load h-chunks

BF16 = ml_dtypes.bfloat16

LAST_RESULT = None    # BassKernelResults of the most recent device run (for test.py)
_NC_CACHE = {}        # capacity -> compiled Bass graph


def _ensure_axon_profile_hook():
    """The container's antenv stub lacks axon_hooks, so trn_boot's NTFF
    profile hook registration silently degrades.  Recreate the module and
    register the ctypes hook so run_bass_kernel_spmd(trace=True) works."""
    if "antenv.axon_hooks" in sys.modules:
        return
    mod = types.ModuleType("antenv.axon_hooks")
    _h = [None]
    mod.set_axon_ntff_profile_hook = lambda h: _h.__setitem__(0, h)
    mod.get_axon_ntff_profile_hook = lambda: _h[0]
    sys.modules["antenv.axon_hooks"] = mod
    try:
        import antenv
        antenv.axon_hooks = mod
    except ImportError:
        pass
    try:
        from trn_agent_boot.trn_boot import _ntff_profile_via_ctypes
        hook = _ntff_profile_via_ctypes("/opt/axon/libaxon_pjrt.so")
        if hook is not None:
            mod.set_axon_ntff_profile_hook(hook)
    except Exception:
        pass


def _route(x, gate_w):
    """Replicates the reference routing exactly (verified vs jax.lax.top_k)."""
    logits = x @ gate_w.T
    m = logits.max(-1, keepdims=True)
    p = np.exp(logits - m)
    p /= p.sum(-1, keepdims=True)
    sel = np.argsort(-p, axis=-1, kind="stable")[:, :TOPK]
    tw = np.take_along_axis(p, sel, axis=-1)
    tw = (tw / tw.sum(-1, keepdims=True)).astype(np.float32)
    return p.astype(np.float32), sel, tw


def _aux_loss(p, sel):
    tpe = np.stack(
        [np.bincount(sel[:, k], minlength=E).astype(np.float32) / np.float32(T)
         for k in range(TOPK)]
    )  # [K, E]
    rppe = p.mean(axis=0, dtype=np.float32)
    return np.float32((tpe * rppe[None, :]).sum() * E)


def _build(c_alloc, c_use):
    """c_alloc: tile/DRAM column allocation (mult of 64, for alignment).
    c_use: columns actually processed (mult of 8, >= max expert count)."""
    import concourse.bass as bass
    import concourse.mybir as mybir
    import concourse.tile as tile
    from concourse import bacc

    f32, bf16 = mybir.dt.float32, mybir.dt.bfloat16
    c_cap = c_alloc
    cch = [(c0, min(512, c_use - c0)) for c0 in range(0, c_use, 512)]

    nc = bacc.Bacc(None, target_bir_lowering=False, debug=False)
    xt_d = nc.declare_dram_parameter("xt", [128, NH, c_cap], bf16, isOutput=False)
    w1_d = nc.declare_dram_parameter("w1t", [NF, 128, NH, 128], bf16, isOutput=False)
    w3_d = nc.declare_dram_parameter("w3t", [NF, 128, NH, 128], bf16, isOutput=False)
    # w2t[hj*2+half, p, f16, h512]  (f = (half*16+f16)*128 + p, h = hj*512 + h512)
    w2_d = nc.declare_dram_parameter("w2t", [2 * NHJ, 128, 16, 512], bf16, isOutput=False)
    wv_d = nc.declare_dram_parameter("wv", [1, c_cap], f32, isOutput=False)
    out_d = nc.declare_dram_parameter("out", [H, c_cap], f32, isOutput=True)

    silu = mybir.ActivationFunctionType.Silu

    with tile.TileContext(nc) as tc:
        with (
            tc.tile_pool(name="big", bufs=1) as big,
            tc.tile_pool(name="tmp", bufs=3) as tmp,
            tc.tile_pool(name="outp", bufs=4) as outp,
        ):
            xt_sb = big.tile([128, NH, c_cap], bf16)
            gt_sb = big.tile([128, NF, c_cap], bf16)
            wbc = big.tile([128, c_cap], f32)

            # ---- Phase A: hmidT tiles [f, c]; g = silu(h1) * h3 -> gt (bf16)
            with (
                tc.tile_pool(name="w13", bufs=2) as w13,
                tc.tile_pool(name="psA", bufs=2, space="PSUM") as psp,
            ):
                first = True
                for fi in range(NF):
                    w1_sl = w13.tile([128, NH, 128], bf16, tag="w1")
                    nc.sync.dma_start(out=w1_sl, in_=w1_d[fi])
                    w3_sl = w13.tile([128, NH, 128], bf16, tag="w3")
                    nc.sync.dma_start(out=w3_sl, in_=w3_d[fi])
                    if first:
                        # after the first weight slices: x chunks + bcast weights.
                        # First c-chunk split by hi-group so the first matmuls
                        # unblock after ~1MB of DMA instead of ~5MB.
                        for icc, (c0, cw) in enumerate(cch):
                            if icc == 0:
                                for hg in range(0, NH, 4):
                                    nc.sync.dma_start(
                                        out=xt_sb[:, hg:hg + 4, c0:c0 + cw],
                                        in_=xt_d[:, hg:hg + 4, c0:c0 + cw])
                            else:
                                nc.sync.dma_start(
                                    out=xt_sb[:, :, c0:c0 + cw],
                                    in_=xt_d[:, :, c0:c0 + cw])
                        wv_ap = wv_d[:]
                        nc.gpsimd.dma_start(
                            out=wbc,
                            in_=bass.AP(tensor=wv_ap.tensor, offset=wv_ap.offset,
                                        ap=[[0, 128], [1, c_cap]]),
                        )
                        first = False
                    for c0, cw in cch:
                        ps1 = psp.tile([128, 512], f32, tag="ps1")
                        ps3 = psp.tile([128, 512], f32, tag="ps3")
                        for hi in range(NH):
                            nc.tensor.matmul(
                                ps1[:, :cw], w1_sl[:, hi, :], xt_sb[:, hi, c0:c0 + cw],
                                start=(hi == 0), stop=(hi == NH - 1),
                            )
                        for hi in range(NH):
                            nc.tensor.matmul(
                                ps3[:, :cw], w3_sl[:, hi, :], xt_sb[:, hi, c0:c0 + cw],
                                start=(hi == 0), stop=(hi == NH - 1),
                            )
                        sil = tmp.tile([128, 512], f32)
                        nc.scalar.activation(sil[:, :cw], ps1[:, :cw], silu)
                        nc.vector.tensor_mul(
                            gt_sb[:, fi, c0:c0 + cw], sil[:, :cw], ps3[:, :cw])

            # ---- Phase B: YT[h, c] = W2T.T @ gT, scaled by combine weight
            with (
                tc.tile_pool(name="w2", bufs=3) as w2p,
                tc.tile_pool(name="psB", bufs=2 * len(cch), space="PSUM") as ps2p,
            ):
                for hj in range(NHJ):
                    w2_h = []
                    for half in range(2):
                        w2_sl = w2p.tile([128, 16, 512], bf16, tag="w2sl")
                        nc.sync.dma_start(out=w2_sl, in_=w2_d[hj * 2 + half])
                        w2_h.append(w2_sl)
                    for hk in range(4):  # 128-row h chunk within hj
                        psy = [ps2p.tile([128, 512], f32, tag="psY", name=f"psy{i}")
                               for i in range(len(cch))]
                        for fi in range(NF):
                            half, f16 = divmod(fi, 16)
                            lhst = w2_h[half][:, f16, hk * 128:(hk + 1) * 128]
                            for icc, (c0, cw) in enumerate(cch):
                                nc.tensor.matmul(
                                    psy[icc][:, :cw], lhst, gt_sb[:, fi, c0:c0 + cw],
                                    start=(fi == 0), stop=(fi == NF - 1),
                                )
                        h0 = (hj * 4 + hk) * 128
                        for icc, (c0, cw) in enumerate(cch):
                            ot = outp.tile([128, 512], f32)
                            nc.vector.tensor_mul(
                                ot[:, :cw], psy[icc][:, :cw], wbc[:, c0:c0 + cw])
                            nc.sync.dma_start(
                                out=out_d[h0:h0 + 128, c0:c0 + cw], in_=ot[:, :cw])
    nc.compile()
    return nc


def _prep_expert(x_bf, w1e, w3e, w2e, idx, wgt, c_cap):
    n = len(idx)

    xt = np.zeros((H, c_cap), BF16)
    xt[:, :n] = x_bf[idx].T
    xt = np.ascontiguousarray(xt.reshape(NH, 128, c_cap).transpose(1, 0, 2))

    w1t = np.ascontiguousarray(
        w1e.astype(BF16).reshape(NF, 128, NH, 128).transpose(0, 3, 2, 1))
    w3t = np.ascontiguousarray(
        w3e.astype(BF16).reshape(NF, 128, NH, 128).transpose(0, 3, 2, 1))
    # w2t[hj, half, p, f16, h] = w2e[hj*512+h, (half*16+f16)*128+p]
    w2t = np.ascontiguousarray(
        w2e.astype(BF16).reshape(NHJ, 512, 2, 16, 128).transpose(0, 2, 4, 3, 1)
    ).reshape(2 * NHJ, 128, 16, 512)

    wv = np.zeros((1, c_cap), np.float32)
    wv[0, :n] = wgt

    return {"xt": xt, "w1t": w1t, "w3t": w3t, "w2t": w2t, "wv": wv}


def kernel(hidden_states, gate_w, w1, w3, w2):
    global LAST_RESULT
    _ensure_axon_profile_hook()
    from concourse.bass_utils import run_bass_kernel_spmd

    hidden_states = np.asarray(hidden_states, np.float32)
    gate_w = np.asarray(gate_w, np.float32)
    w1 = np.asarray(w1, np.float32)
    w3 = np.asarray(w3, np.float32)
    w2 = np.asarray(w2, np.float32)

    x = hidden_states.reshape(T, H)
    p, sel, tw = _route(x, gate_w)
    aux = _aux_loss(p, sel)

    idxs, wgts = [], []
    for e in range(E):
        hit = sel == e                      # [T, K]
        tok = np.nonzero(hit.any(axis=1))[0]
        k_of = hit[tok].argmax(axis=1)
        idxs.append(tok)
        wgts.append(tw[tok, k_of])
    maxcnt = max(len(i) for i in idxs)
    c_alloc = max(128, -(-maxcnt // 64) * 64)
    c_use = max(128, -(-maxcnt // 8) * 8)

    x_bf = x.astype(BF16)
    in_maps = [
        _prep_expert(x_bf, w1[e], w3[e], w2[e], idxs[e], wgts[e], c_alloc)
        for e in range(E)
    ]

    key = (c_alloc, c_use)
    if key not in _NC_CACHE:
        _NC_CACHE[key] = _build(c_alloc, c_use)
    nc = _NC_CACHE[key]

    trace = bool(os.environ.get("KERNEL_TRACE"))
    LAST_RESULT = run_bass_kernel_spmd(
        nc, in_maps, core_ids=list(range(E)),
        trace=trace, trace_cores=list(range(E)) if trace else None,
    )

    out = np.zeros((T, H), np.float32)
    for e in range(E):
        out[idxs[e]] += LAST_RESULT.results[e]["out"][:, : len(idxs[e])].T
    return out.reshape(B, S, H), aux


# revision 10
# speedup vs baseline: 1.1912x; 1.1912x over previous
"""ArcticMoE top-2 routing MoE — Trainium2 Bass kernel, expert-parallel over 8 cores.

Strategy:
  - Host computes the (tiny) router: logits = x @ gate_w.T, softmax, top-2,
    normalized combine weights, aux loss.  This is 0.03% of total FLOPs.
  - Expert parallelism: core e holds expert e's weights (pre-transposed/tiled
    to the exact SBUF layouts the matmuls need, cast to bf16 on host).
    Core e receives only the tokens routed to expert e (gathered, transposed),
    padded to a common capacity C so all 8 cores run one SPMD graph.
  - Device per core: hmidT = silu(W1T.T@XT) * (W3T.T@XT)  (tiles [f,c]),
    YT = W2T.T @ gT (tiles [h,c]) scaled by per-token combine weight.
  - Host scatter-adds each expert's [count, H] result into the [T, H] output.
"""

import os
import sys
import types

import numpy as np
import ml_dtypes

B, S, H, F, E, TOPK = 2, 2048, 2048, 4096, 8, 2
T = B * S
NH = H // 128   # 16 h-chunks
NF = F // 128   # 32 f-chunks
NHJ = H // 512  # 4 w2-load h-chunks

BF16 = ml_dtypes.bfloat16

LAST_RESULT = None    # BassKernelResults of the most recent device run (for test.py)
_NC_CACHE = {}        # capacity -> compiled Bass graph


def _ensure_axon_profile_hook():
    """The container's antenv stub lacks axon_hooks, so trn_boot's NTFF
    profile hook registration silently degrades.  Recreate the module and
    register the ctypes hook so run_bass_kernel_spmd(trace=True) works."""
    if "antenv.axon_hooks" in sys.modules:
        return
    mod = types.ModuleType("antenv.axon_hooks")
    _h = [None]
    mod.set_axon_ntff_profile_hook = lambda h: _h.__setitem__(0, h)
    mod.get_axon_ntff_profile_hook = lambda: _h[0]
    sys.modules["antenv.axon_hooks"] = mod
    try:
        import antenv
        antenv.axon_hooks = mod
    except ImportError:
        pass
    try:
        from trn_agent_boot.trn_boot import _ntff_profile_via_ctypes
        hook = _ntff_profile_via_ctypes("/opt/axon/libaxon_pjrt.so")
        if hook is not None:
            mod.set_axon_ntff_profile_hook(hook)
    except Exception:
        pass


def _route(x, gate_w):
    """Replicates the reference routing exactly (verified vs jax.lax.top_k)."""
    logits = x @ gate_w.T
    m = logits.max(-1, keepdims=True)
    p = np.exp(logits - m)
    p /= p.sum(-1, keepdims=True)
    sel = np.argsort(-p, axis=-1, kind="stable")[:, :TOPK]
    tw = np.take_along_axis(p, sel, axis=-1)
    tw = (tw / tw.sum(-1, keepdims=True)).astype(np.float32)
    return p.astype(np.float32), sel, tw


def _aux_loss(p, sel):
    tpe = np.stack(
        [np.bincount(sel[:, k], minlength=E).astype(np.float32) / np.float32(T)
         for k in range(TOPK)]
    )  # [K, E]
    rppe = p.mean(axis=0, dtype=np.float32)
    return np.float32((tpe * rppe[None, :]).sum() * E)


def _build(c_alloc, c_use):
    """c_alloc: tile/DRAM column allocation (mult of 64, for alignment).
    c_use: columns actually processed (mult of 8, >= max expert count)."""
    import concourse.bass as bass
    import concourse.mybir as mybir
    import concourse.tile as tile
    from concourse import bacc

    f32, bf16 = mybir.dt.float32, mybir.dt.bfloat16
    c_cap = c_alloc
    cch = [(c0, min(512, c_use - c0)) for c0 in range(0, c_use, 512)]

    nc = bacc.Bacc(None, target_bir_lowering=False, debug=False)
    xt_d = nc.declare_dram_parameter("xt", [128, NH, c_cap], bf16, isOutput=False)
    w1_d = nc.declare_dram_parameter("w1t", [NF, 128, NH, 128], bf16, isOutput=False)
    w3_d = nc.declare_dram_parameter("w3t", [NF, 128, NH, 128], bf16, isOutput=False)
    # w2t[hj*2+half, p, f16, h512]  (f = (half*16+f16)*128 + p, h = hj*512 + h512)
    w2_d = nc.declare_dram_parameter("w2t", [2 * NHJ, 128, 16, 512], bf16, isOutput=False)
    wv_d = nc.declare_dram_parameter("wv", [1, c_cap], f32, isOutput=False)
    out_d = nc.declare_dram_parameter("out", [H, c_cap], f32, isOutput=True)

    silu = mybir.ActivationFunctionType.Silu

    with tile.TileContext(nc) as tc:
        with (
            tc.tile_pool(name="big", bufs=1) as big,
            tc.tile_pool(name="tmp", bufs=3) as tmp,
            tc.tile_pool(name="outp", bufs=4) as outp,
        ):
            xt_sb = big.tile([128, NH, c_cap], bf16)
            gt_sb = big.tile([128, NF, c_cap], bf16)
            wbc = big.tile([128, c_cap], f32)

            # ---- Phase A: hmidT tiles [f, c]; g = silu(h1) * h3 -> gt (bf16)
            with (
                tc.tile_pool(name="w13", bufs=2) as w13,
                tc.tile_pool(name="psA", bufs=2, space="PSUM") as psp,
            ):
                first = True
                for fi in range(NF):
                    w1_sl = w13.tile([128, NH, 128], bf16, tag="w1")
                    w3_sl = w13.tile([128, NH, 128], bf16, tag="w3")
                    if first:
                        # Startup staging: interleave the first w1 slice and the
                        # first xt c-chunk in hi-group pieces so the first
                        # matmul unblocks after ~0.7MB of DMA; defer w3 and the
                        # remaining xt chunks behind them.
                        c0, cw = cch[0]
                        for hg in range(0, NH, 4):
                            nc.sync.dma_start(
                                out=w1_sl[:, hg:hg + 4, :], in_=w1_d[fi, :, hg:hg + 4, :])
                            nc.sync.dma_start(
                                out=xt_sb[:, hg:hg + 4, c0:c0 + cw],
                                in_=xt_d[:, hg:hg + 4, c0:c0 + cw])
                        nc.sync.dma_start(out=w3_sl, in_=w3_d[fi])
                        for c0, cw in cch[1:]:
                            nc.sync.dma_start(
                                out=xt_sb[:, :, c0:c0 + cw], in_=xt_d[:, :, c0:c0 + cw])
                        wv_ap = wv_d[:]
                        nc.gpsimd.dma_start(
                            out=wbc,
                            in_=bass.AP(tensor=wv_ap.tensor, offset=wv_ap.offset,
                                        ap=[[0, 128], [1, c_cap]]),
                        )
                        first = False
                    else:
                        nc.sync.dma_start(out=w1_sl, in_=w1_d[fi])
                        nc.sync.dma_start(out=w3_sl, in_=w3_d[fi])
                    for c0, cw in cch:
                        ps1 = psp.tile([128, 512], f32, tag="ps1")
                        ps3 = psp.tile([128, 512], f32, tag="ps3")
                        for hi in range(NH):
                            nc.tensor.matmul(
                                ps1[:, :cw], w1_sl[:, hi, :], xt_sb[:, hi, c0:c0 + cw],
                                start=(hi == 0), stop=(hi == NH - 1),
                            )
                        for hi in range(NH):
                            nc.tensor.matmul(
                                ps3[:, :cw], w3_sl[:, hi, :], xt_sb[:, hi, c0:c0 + cw],
                                start=(hi == 0), stop=(hi == NH - 1),
                            )
                        sil = tmp.tile([128, 512], f32)
                        nc.scalar.activation(sil[:, :cw], ps1[:, :cw], silu)
                        nc.vector.tensor_mul(
                            gt_sb[:, fi, c0:c0 + cw], sil[:, :cw], ps3[:, :cw])

            # ---- Phase B: YT[h, c] = W2T.T @ gT, scaled by combine weight
            with (
                tc.tile_pool(name="w2", bufs=3) as w2p,
                tc.tile_pool(name="psB", bufs=2 * len(cch), space="PSUM") as ps2p,
            ):
                for hj in range(NHJ):
                    w2_h = []
                    for half in range(2):
                        w2_sl = w2p.tile([128, 16, 512], bf16, tag="w2sl")
                        nc.sync.dma_start(out=w2_sl, in_=w2_d[hj * 2 + half])
                        w2_h.append(w2_sl)
                    for hk in range(4):  # 128-row h chunk within hj
                        psy = [ps2p.tile([128, 512], f32, tag="psY", name=f"psy{i}")
                               for i in range(len(cch))]
                        for fi in range(NF):
                            half, f16 = divmod(fi, 16)
                            lhst = w2_h[half][:, f16, hk * 128:(hk + 1) * 128]
                            for icc, (c0, cw) in enumerate(cch):
                                nc.tensor.matmul(
                                    psy[icc][:, :cw], lhst, gt_sb[:, fi, c0:c0 + cw],
                                    start=(fi == 0), stop=(fi == NF - 1),
                                )
                        h0 = (hj * 4 + hk) * 128
                        for icc, (c0, cw) in enumerate(cch):
                            ot = outp.tile([128, 512], f32)
                            nc.vector.tensor_mul(
                                ot[:, :cw], psy[icc][:, :cw], wbc[:, c0:c0 + cw])
                            nc.sync.dma_start(
                                out=out_d[h0:h0 + 128, c0:c0 + cw], in_=ot[:, :cw])
    nc.compile()
    return nc


def _prep_expert(x_bf, w1e, w3e, w2e, idx, wgt, c_cap):
    n = len(idx)

    xt = np.zeros((H, c_cap), BF16)
    xt[:, :n] = x_bf[idx].T
    xt = np.ascontiguousarray(xt.reshape(NH, 128, c_cap).transpose(1, 0, 2))

    w1t = np.ascontiguousarray(
        w1e.astype(BF16).reshape(NF, 128, NH, 128).transpose(0, 3, 2, 1))
    w3t = np.ascontiguousarray(
        w3e.astype(BF16).reshape(NF, 128, NH, 128).transpose(0, 3, 2, 1))
    # w2t[hj, half, p, f16, h] = w2e[hj*512+h, (half*16+f16)*128+p]
    w2t = np.ascontiguousarray(
        w2e.astype(BF16).reshape(NHJ, 512, 2, 16, 128).transpose(0, 2, 4, 3, 1)
    ).reshape(2 * NHJ, 128, 16, 512)

    wv = np.zeros((1, c_cap), np.float32)
    wv[0, :n] = wgt

    return {"xt": xt, "w1t": w1t, "w3t": w3t, "w2t": w2t, "wv": wv}


def kernel(hidden_states, gate_w, w1, w3, w2):
    global LAST_RESULT
    _ensure_axon_profile_hook()
    from concourse.bass_utils import run_bass_kernel_spmd

    hidden_states = np.asarray(hidden_states, np.float32)
    gate_w = np.asarray(gate_w, np.float32)
    w1 = np.asarray(w1, np.float32)
    w3 = np.asarray(w3, np.float32)
    w2 = np.asarray(w2, np.float32)

    x = hidden_states.reshape(T, H)
    p, sel, tw = _route(x, gate_w)
    aux = _aux_loss(p, sel)

    idxs, wgts = [], []
    for e in range(E):
        hit = sel == e                      # [T, K]
        tok = np.nonzero(hit.any(axis=1))[0]
        k_of = hit[tok].argmax(axis=1)
        idxs.append(tok)
        wgts.append(tw[tok, k_of])
    maxcnt = max(len(i) for i in idxs)
    c_alloc = max(128, -(-maxcnt // 64) * 64)
    c_use = max(128, -(-maxcnt // 8) * 8)

    x_bf = x.astype(BF16)
    in_maps = [
        _prep_expert(x_bf, w1[e], w3[e], w2[e], idxs[e], wgts[e], c_alloc)
        for e in range(E)
    ]

    key = (c_alloc, c_use)
    if key not in _NC_CACHE:
        _NC_CACHE[key] = _build(c_alloc, c_use)
    nc = _NC_CACHE[key]

    trace = bool(os.environ.get("KERNEL_TRACE"))
    tcores = os.environ.get("KERNEL_TRACE_CORES")
    tcores = [int(c) for c in tcores.split(",")] if tcores else list(range(E))

    last_err = None
    for attempt in range(3):
        try:
            LAST_RESULT = run_bass_kernel_spmd(
                nc, in_maps, core_ids=list(range(E)),
                trace=trace, trace_cores=tcores if trace else None,
            )
            outs = [LAST_RESULT.results[e]["out"] for e in range(E)]
            if all(np.isfinite(o).all() for o in outs):
                break
            last_err = RuntimeError("non-finite device output")
        except Exception as err:  # transient NRT/device errors
            last_err = err
            import time
            time.sleep(5)
    else:
        raise last_err

    out = np.zeros((T, H), np.float32)
    for e in range(E):
        out[idxs[e]] += outs[e][:, : len(idxs[e])].T
    return out.reshape(B, S, H), aux


# revision 13
# speedup vs baseline: 1.1934x; 1.0018x over previous
"""ArcticMoE top-2 routing MoE — Trainium2 Bass kernel, expert-parallel over 8 cores.

Strategy:
  - Host computes the (tiny) router: logits = x @ gate_w.T, softmax, top-2,
    normalized combine weights, aux loss.  This is 0.03% of total FLOPs.
  - Expert parallelism: core e holds expert e's weights (pre-transposed/tiled
    to the exact SBUF layouts the matmuls need, cast to bf16 on host).
    Core e receives only the tokens routed to expert e (gathered, transposed),
    padded to a common capacity C so all 8 cores run one SPMD graph.
  - Device per core: hmidT = silu(W1T.T@XT) * (W3T.T@XT)  (tiles [f,c]),
    YT = W2T.T @ gT (tiles [h,c]) scaled by per-token combine weight.
  - Host scatter-adds each expert's [count, H] result into the [T, H] output.
"""

import os
import sys
import types

import numpy as np
import ml_dtypes

B, S, H, F, E, TOPK = 2, 2048, 2048, 4096, 8, 2
T = B * S
NH = H // 128   # 16 h-chunks
NF = F // 128   # 32 f-chunks
NHJ = H // 512  # 4 w2-load h-chunks

BF16 = ml_dtypes.bfloat16

LAST_RESULT = None    # BassKernelResults of the most recent device run (for test.py)
_NC_CACHE = {}        # capacity -> compiled Bass graph


def _ensure_axon_profile_hook():
    """The container's antenv stub lacks axon_hooks, so trn_boot's NTFF
    profile hook registration silently degrades.  Recreate the module and
    register the ctypes hook so run_bass_kernel_spmd(trace=True) works."""
    if "antenv.axon_hooks" in sys.modules:
        return
    mod = types.ModuleType("antenv.axon_hooks")
    _h = [None]
    mod.set_axon_ntff_profile_hook = lambda h: _h.__setitem__(0, h)
    mod.get_axon_ntff_profile_hook = lambda: _h[0]
    sys.modules["antenv.axon_hooks"] = mod
    try:
        import antenv
        antenv.axon_hooks = mod
    except ImportError:
        pass
    try:
        from trn_agent_boot.trn_boot import _ntff_profile_via_ctypes
        hook = _ntff_profile_via_ctypes("/opt/axon/libaxon_pjrt.so")
        if hook is not None:
            mod.set_axon_ntff_profile_hook(hook)
    except Exception:
        pass


def _route(x, gate_w):
    """Replicates the reference routing exactly (verified vs jax.lax.top_k)."""
    logits = x @ gate_w.T
    m = logits.max(-1, keepdims=True)
    p = np.exp(logits - m)
    p /= p.sum(-1, keepdims=True)
    sel = np.argsort(-p, axis=-1, kind="stable")[:, :TOPK]
    tw = np.take_along_axis(p, sel, axis=-1)
    tw = (tw / tw.sum(-1, keepdims=True)).astype(np.float32)
    return p.astype(np.float32), sel, tw


def _aux_loss(p, sel):
    tpe = np.stack(
        [np.bincount(sel[:, k], minlength=E).astype(np.float32) / np.float32(T)
         for k in range(TOPK)]
    )  # [K, E]
    rppe = p.mean(axis=0, dtype=np.float32)
    return np.float32((tpe * rppe[None, :]).sum() * E)


def _build(c_alloc, c_use):
    """c_alloc: tile/DRAM column allocation (mult of 64, for alignment).
    c_use: columns actually processed (mult of 8, >= max expert count)."""
    import concourse.bass as bass
    import concourse.mybir as mybir
    import concourse.tile as tile
    from concourse import bacc

    f32, bf16 = mybir.dt.float32, mybir.dt.bfloat16
    c_cap = c_alloc
    cch = [(c0, min(512, c_use - c0)) for c0 in range(0, c_use, 512)]

    nc = bacc.Bacc(None, target_bir_lowering=False, debug=False)
    xt_d = nc.declare_dram_parameter("xt", [128, NH, c_cap], bf16, isOutput=False)
    w1_d = nc.declare_dram_parameter("w1t", [NF, 128, NH, 128], bf16, isOutput=False)
    w3_d = nc.declare_dram_parameter("w3t", [NF, 128, NH, 128], bf16, isOutput=False)
    # w2t[hj*2+half, p, f16, h512]  (f = (half*16+f16)*128 + p, h = hj*512 + h512)
    w2_d = nc.declare_dram_parameter("w2t", [2 * NHJ, 128, 16, 512], bf16, isOutput=False)
    wv_d = nc.declare_dram_parameter("wv", [1, c_cap], f32, isOutput=False)
    out_d = nc.declare_dram_parameter("out", [H, c_cap], f32, isOutput=True)

    silu = mybir.ActivationFunctionType.Silu

    # SBUF/partition budget: xt 32B/col + gt 64B/col + wbc 4B/col + pools.
    # At c_alloc <= 1152 the full-buffered pools fit under Tile's 192KB limit;
    # beyond that, shrink double-buffering to stay correct for skewed routings.
    roomy = c_alloc <= 1152
    with tile.TileContext(nc) as tc:
        with (
            tc.tile_pool(name="big", bufs=1) as big,
            tc.tile_pool(name="tmp", bufs=3 if roomy else 2) as tmp,
            tc.tile_pool(name="outp", bufs=4 if roomy else 2) as outp,
        ):
            xt_sb = big.tile([128, NH, c_cap], bf16)
            gt_sb = big.tile([128, NF, c_cap], bf16)
            wbc = big.tile([128, c_cap], f32)

            # ---- Phase A: hmidT tiles [f, c]; g = silu(h1) * h3 -> gt (bf16)
            with (
                tc.tile_pool(name="w13", bufs=2) as w13,
                tc.tile_pool(name="psA", bufs=3, space="PSUM") as psp,
            ):
                first = True
                for fi in range(NF):
                    w1_sl = w13.tile([128, NH, 128], bf16, tag="w1")
                    w3_sl = w13.tile([128, NH, 128], bf16, tag="w3")
                    if first:
                        # Startup staging: interleave the first w1 slice and the
                        # first xt c-chunk in hi-group pieces so the first
                        # matmul unblocks after ~0.7MB of DMA; defer w3 and the
                        # remaining xt chunks behind them.
                        c0, cw = cch[0]
                        for hg in range(0, NH, 4):
                            nc.sync.dma_start(
                                out=w1_sl[:, hg:hg + 4, :], in_=w1_d[fi, :, hg:hg + 4, :])
                            nc.sync.dma_start(
                                out=xt_sb[:, hg:hg + 4, c0:c0 + cw],
                                in_=xt_d[:, hg:hg + 4, c0:c0 + cw])
                        nc.sync.dma_start(out=w3_sl, in_=w3_d[fi])
                        for c0, cw in cch[1:]:
                            nc.sync.dma_start(
                                out=xt_sb[:, :, c0:c0 + cw], in_=xt_d[:, :, c0:c0 + cw])
                        wv_ap = wv_d[:]
                        nc.gpsimd.dma_start(
                            out=wbc,
                            in_=bass.AP(tensor=wv_ap.tensor, offset=wv_ap.offset,
                                        ap=[[0, 128], [1, c_cap]]),
                        )
                        first = False
                    else:
                        nc.sync.dma_start(out=w1_sl, in_=w1_d[fi])
                        nc.sync.dma_start(out=w3_sl, in_=w3_d[fi])
                    for c0, cw in cch:
                        ps1 = psp.tile([128, 512], f32, tag="ps1")
                        ps3 = psp.tile([128, 512], f32, tag="ps3")
                        for hi in range(NH):
                            nc.tensor.matmul(
                                ps1[:, :cw], w1_sl[:, hi, :], xt_sb[:, hi, c0:c0 + cw],
                                start=(hi == 0), stop=(hi == NH - 1),
                            )
                        for hi in range(NH):
                            nc.tensor.matmul(
                                ps3[:, :cw], w3_sl[:, hi, :], xt_sb[:, hi, c0:c0 + cw],
                                start=(hi == 0), stop=(hi == NH - 1),
                            )
                        sil = tmp.tile([128, 512], f32)
                        nc.scalar.activation(sil[:, :cw], ps1[:, :cw], silu)
                        nc.vector.tensor_mul(
                            gt_sb[:, fi, c0:c0 + cw], sil[:, :cw], ps3[:, :cw])

            # ---- Phase B: YT[h, c] = W2T.T @ gT, scaled by combine weight
            with (
                tc.tile_pool(name="w2", bufs=3 if roomy else 2) as w2p,
                tc.tile_pool(name="psB", bufs=2 * len(cch), space="PSUM") as ps2p,
            ):
                for hj in range(NHJ):
                    w2_h = []
                    for half in range(2):
                        w2_sl = w2p.tile([128, 16, 512], bf16, tag="w2sl")
                        nc.sync.dma_start(out=w2_sl, in_=w2_d[hj * 2 + half])
                        w2_h.append(w2_sl)
                    for hk in range(4):  # 128-row h chunk within hj
                        psy = [ps2p.tile([128, 512], f32, tag="psY", name=f"psy{i}")
                               for i in range(len(cch))]
                        for fi in range(NF):
                            half, f16 = divmod(fi, 16)
                            lhst = w2_h[half][:, f16, hk * 128:(hk + 1) * 128]
                            for icc, (c0, cw) in enumerate(cch):
                                nc.tensor.matmul(
                                    psy[icc][:, :cw], lhst, gt_sb[:, fi, c0:c0 + cw],
                                    start=(fi == 0), stop=(fi == NF - 1),
                                )
                        h0 = (hj * 4 + hk) * 128
                        for icc, (c0, cw) in enumerate(cch):
                            ot = outp.tile([128, 512], f32)
                            nc.vector.tensor_mul(
                                ot[:, :cw], psy[icc][:, :cw], wbc[:, c0:c0 + cw])
                            nc.sync.dma_start(
                                out=out_d[h0:h0 + 128, c0:c0 + cw], in_=ot[:, :cw])
    nc.compile()
    return nc


def _prep_expert(x_bf, w1e, w3e, w2e, idx, wgt, c_cap):
    n = len(idx)

    xt = np.zeros((H, c_cap), BF16)
    xt[:, :n] = x_bf[idx].T
    xt = np.ascontiguousarray(xt.reshape(NH, 128, c_cap).transpose(1, 0, 2))

    w1t = np.ascontiguousarray(
        w1e.astype(BF16).reshape(NF, 128, NH, 128).transpose(0, 3, 2, 1))
    w3t = np.ascontiguousarray(
        w3e.astype(BF16).reshape(NF, 128, NH, 128).transpose(0, 3, 2, 1))
    # w2t[hj, half, p, f16, h] = w2e[hj*512+h, (half*16+f16)*128+p]
    w2t = np.ascontiguousarray(
        w2e.astype(BF16).reshape(NHJ, 512, 2, 16, 128).transpose(0, 2, 4, 3, 1)
    ).reshape(2 * NHJ, 128, 16, 512)

    wv = np.zeros((1, c_cap), np.float32)
    wv[0, :n] = wgt

    return {"xt": xt, "w1t": w1t, "w3t": w3t, "w2t": w2t, "wv": wv}


def kernel(hidden_states, gate_w, w1, w3, w2):
    global LAST_RESULT
    _ensure_axon_profile_hook()
    from concourse.bass_utils import run_bass_kernel_spmd

    hidden_states = np.asarray(hidden_states, np.float32)
    gate_w = np.asarray(gate_w, np.float32)
    w1 = np.asarray(w1, np.float32)
    w3 = np.asarray(w3, np.float32)
    w2 = np.asarray(w2, np.float32)

    x = hidden_states.reshape(T, H)
    p, sel, tw = _route(x, gate_w)
    aux = _aux_loss(p, sel)

    idxs, wgts = [], []
    for e in range(E):
        hit = sel == e                      # [T, K]
        tok = np.nonzero(hit.any(axis=1))[0]
        k_of = hit[tok].argmax(axis=1)
        idxs.append(tok)
        wgts.append(tw[tok, k_of])
    maxcnt = max(len(i) for i in idxs)
    c_alloc = max(128, -(-maxcnt // 64) * 64)
    c_use = max(128, -(-maxcnt // 8) * 8)

    x_bf = x.astype(BF16)
    in_maps = [
        _prep_expert(x_bf, w1[e], w3[e], w2[e], idxs[e], wgts[e], c_alloc)
        for e in range(E)
    ]

    key = (c_alloc, c_use)
    if key not in _NC_CACHE:
        _NC_CACHE[key] = _build(c_alloc, c_use)
    nc = _NC_CACHE[key]

    trace = bool(os.environ.get("KERNEL_TRACE"))
    tcores = os.environ.get("KERNEL_TRACE_CORES")
    tcores = [int(c) for c in tcores.split(",")] if tcores else list(range(E))

    last_err = None
    for attempt in range(3):
        try:
            LAST_RESULT = run_bass_kernel_spmd(
                nc, in_maps, core_ids=list(range(E)),
                trace=trace, trace_cores=tcores if trace else None,
            )
            outs = [LAST_RESULT.results[e]["out"] for e in range(E)]
            if all(np.isfinite(o).all() for o in outs):
                break
            last_err = RuntimeError("non-finite device output")
        except Exception as err:  # transient NRT/device errors
            last_err = err
            import time
            time.sleep(5)
    else:
        raise last_err

    out = np.zeros((T, H), np.float32)
    for e in range(E):
        out[idxs[e]] += outs[e][:, : len(idxs[e])].T
    return out.reshape(B, S, H), aux
